# revision 2
# baseline (speedup 1.0000x reference)
"""Continuous Game-of-Life Trainium2 kernel (product-form, 2-sigmoid).

Reference computation (per batch image, cyclic 3x3 stencil):
    around = 8-neighbor sum of x (torus wrap), u = 10*around
    survive = sigmoid(u-15) * sigmoid(35-u)
    birth   = sigmoid(u-25) * sigmoid(35-u)
    out     = x*survive + (1-x)*birth

Math used here (max abs err ~5e-5 vs reference, fp64):
    E1 := sigmoid(10 - |u-25|)        # == survive (err <= sigmoid(-10))
    E2 := sigmoid(u-25)
    birth == E1*E2 (err ~5e-5), so
    out = E1 * (x + E2 - x*E2)

This needs only TWO sigmoid passes on the Scalar engine (the baseline
three-sigmoid form is ScalarE-bound at ~196us busy).  The remaining
work is spread to keep every engine under the ~4.4us/strip DMA floor:
  - TensorE: 8-neighbor sum via banded matmuls (as before).
  - abs pass w = |around-2.5|: split ScalarE (Abs activation, ~30%) /
    VectorE (tensor_scalar add+abs_max, ~70%; PSUM source runs 1x).
  - blend t = x + E2 - x*E2: one fused custom-DVE op (BLEND1_ANT).
  - out = E1*t: split VectorE (2x fp16) / GpSimd.
  - DMA in fp32->fp16 (SWDGE cast), out fp16.

Sharding: data-parallel over batch: 16 images -> 8 cores x 2 images.
Torus wrap is per-image so there is no cross-core halo.
"""

import numpy as np

B, H, W = 16, 2048, 2048
N_CORES = 8
B_PER = B // N_CORES  # 2 images per core
STRIDE = 126  # output rows per strip (128 input rows incl. halos)
N_STRIPS = (H + STRIDE - 1) // STRIDE  # 17
NBANKS = W // 512  # PSUM banks per strip

# work-split knobs (elements of the 2048-wide free dim)
ABS_ACT_W = 640  # abs columns done on ScalarE (rest on VectorE)
MUL_DVE_W = 1280  # final-mul columns done on VectorE (rest on GpSimd)

_cached_nc = None
_blend1_op = None


def _register_blend1():
    """Register the fused blend op out = in0 + in1 - in0*in1 at runtime.

    Same mechanism as editing dve_ops.py (the per-NEFF uop table is
    generated at compile time from OPS); the sha is computed here so the
    pin always matches this interpreter's lowering.
    """
    global _blend1_op
    if _blend1_op is not None:
        return _blend1_op
    from concourse import dve_ops
    from concourse.dve_spec import Spec, Src0, Src1, lower
    from concourse.dve_uop import DveOpSpec

    name = "BLEND1_ANT"
    if name in dve_ops._SUB_OPCODE_FOR_NAME:
        _blend1_op = next(op for op in dve_ops.OPS if op.name == name)
        return _blend1_op
    spec = Spec(
        body=Src0 + Src1 - Src0 * Src1,
        reference=lambda in0, in1, s0, s1, imm2: in0 + in1 - in0 * in1,
    )
    shas = {
        ver: DveOpSpec(name=name, opcode=0, uops=lower(spec, ver=ver), rd1_en=True).sha(
            ver
        )
        for ver in ("v3", "v4")
    }
    op = dve_ops.DveOp(name, spec, subdim=False, uops_sha=shas)
    row = dve_ops._CUSTOM_DVE_ROW_BASE + len(dve_ops.OPS)
    assert row < 0x20
    dve_ops.OPS.append(op)
    dve_ops._SUB_OPCODE_FOR_NAME[name] = row
    dve_ops.CUSTOM_DVE_SPECS[name] = spec
    _blend1_op = op
    return op


def _band_matrices(m, dtype=np.float16):
    """[m+2, m] stationary operands for the vertical taps.

    Tile layout: partitions 0..m-1 hold image rows r0..r0+m-1 (the cells),
    partition m holds the bottom halo row r0+m, partition m+1 holds the top
    halo row r0-1.  For output row p the vertical neighbors are partitions
    p-1 (or m+1 when p==0) and p+1.

    m0[k, p] = 1 for the two vertical neighbors (no center),
    m1[k, p] = 1 for the full 3-tap (used on the column-shifted views).
    """
    m0 = np.zeros((m + 2, m), dtype)
    m1 = np.zeros((m + 2, m), dtype)
    for p in range(m):
        up = m + 1 if p == 0 else p - 1
        m0[up, p] = 1.0
        m0[p + 1, p] = 1.0
        m1[up, p] = 1.0
        m1[p, p] = 1.0
        m1[p + 1, p] = 1.0
    return m0, m1


def _build(b_per=B_PER, h=H, w=W, stride=STRIDE):
    global _cached_nc
    if _cached_nc is not None and (b_per, h, w, stride) == (B_PER, H, W, STRIDE):
        return _cached_nc

    import concourse.mybir as mybir
    from concourse.bacc import Bacc
    from concourse.tile import TileContext

    blend1 = _register_blend1()

    B_PER_, H_, W_, STRIDE_ = b_per, h, w, stride
    N_STRIPS_ = (H_ + STRIDE_ - 1) // STRIDE_
    NBANKS_ = W_ // 512
    KROWS = STRIDE_ + 2

    f32 = mybir.dt.float32
    f16 = mybir.dt.float16
    Sig = mybir.ActivationFunctionType.Sigmoid
    AbsF = mybir.ActivationFunctionType.Abs
    Add = mybir.AluOpType.add
    AbsMax = mybir.AluOpType.abs_max

    nc = Bacc(trn_type="TRN2")
    x_d = nc.dram_tensor("x", [B_PER_, H_, W_], f32, kind="ExternalInput")
    y_d = nc.dram_tensor("y", [B_PER_, H_, W_], f16, kind="ExternalOutput")

    consts = {}
    for m in sorted({STRIDE_, H_ - STRIDE_ * (N_STRIPS_ - 1)}):
        m0_np, m1_np = _band_matrices(m)
        consts[m] = (
            nc.inline_tensor(m0_np, f"m0_const_{m}"),
            nc.inline_tensor(m1_np, f"m1_const_{m}"),
        )

    with TileContext(nc) as tc:
        with (
            tc.tile_pool(name="wpool", bufs=1) as wpool,
            tc.tile_pool(name="xpool", bufs=6) as xpool,
            tc.tile_pool(name="apool", bufs=4) as apool,
            tc.tile_pool(name="spool", bufs=4) as spool,
            tc.tile_pool(name="tpool", bufs=4) as tpool,
            tc.tile_pool(name="opool", bufs=6) as opool,
            tc.tile_pool(name="ppool", bufs=2, space="PSUM") as ppool,
        ):
            bands = {}
            for m, (m0_d, m1_d) in consts.items():
                m0 = wpool.tile([m + 2, m], f16, name=f"m0_{m}")
                m1 = wpool.tile([m + 2, m], f16, name=f"m1_{m}")
                nc.sync.dma_start(out=m0[:], in_=m0_d[:])
                nc.sync.dma_start(out=m1[:], in_=m1_d[:])
                bands[m] = (m0, m1)

            # activation biases must be [128,1] APs, not immediates
            bm25 = wpool.tile([128, 1], f32)
            bp10 = wpool.tile([128, 1], f32)
            nc.vector.memset(bm25[:], -25.0)
            nc.vector.memset(bp10[:], 10.0)

            for b in range(B_PER_):
                for t in range(N_STRIPS_):
                    r0 = t * STRIDE_
                    M = min(STRIDE_, H_ - r0)  # output rows this strip
                    k = M + 2
                    m0, m1 = bands[M]

                    # fp16 tile, partitions 0..M-1 = cells (rows r0..),
                    # partition M = bottom halo, M+1 = top halo.  gpsimd
                    # (SWDGE) DMA casts fp32->fp16 in flight.
                    xt = xpool.tile([KROWS, W_], f16, tag="xt")
                    if r0 + M < H_:
                        nc.gpsimd.dma_start(
                            out=xt[0 : M + 1, :], in_=x_d[b, r0 : r0 + M + 1, :]
                        )
                    else:
                        # last strip: bottom halo wraps to row 0
                        nc.gpsimd.dma_start(out=xt[0:M, :], in_=x_d[b, r0:H_, :])
                        nc.gpsimd.dma_start(out=xt[M : M + 1, :], in_=x_d[b, 0:1, :])
                    rtop = (r0 - 1) % H_
                    nc.gpsimd.dma_start(
                        out=xt[M + 1 : M + 2, :], in_=x_d[b, rtop : rtop + 1, :]
                    )

                    ps = ppool.tile([STRIDE_, W_], f32, tag="ps")
                    m0s = m0[:k, :M]
                    m1s = m1[:k, :M]

                    # Pre-touch: a 1x1 matmul absorbs the PSUM-release wait
                    # (Matmult carries at most ONE sync wait; without this,
                    # wait-merging couples strip t to strip t-1's consumers
                    # and serializes PE behind ACT/DVE).
                    nc.tensor.matmul(
                        ps[:1, 0:1], bm25[:1, :1], bm25[:1, :1],
                        start=True, stop=True,
                    )

                    # around = sum of 8 neighbors, accumulated in PSUM.
                    for nb in range(NBANKS_):
                        c0 = nb * 512
                        c1 = c0 + 512
                        # center column, vertical neighbors only
                        nc.tensor.matmul(
                            ps[:M, c0:c1], m0s, xt[:k, c0:c1],
                            start=True, stop=False,
                        )
                        # left-neighbor column: out col j += band @ x col j-1
                        if nb == 0:
                            nc.tensor.matmul(
                                ps[:M, 1:512], m1s, xt[:k, 0:511],
                                start=False, stop=False,
                            )
                            nc.tensor.matmul(
                                ps[:M, 0:1], m1s, xt[:k, W_ - 1 : W_],
                                start=False, stop=False,
                            )
                        else:
                            nc.tensor.matmul(
                                ps[:M, c0:c1], m1s, xt[:k, c0 - 1 : c1 - 1],
                                start=False, stop=False,
                            )
                        # right-neighbor column: out col j += band @ x col j+1
                        if nb == NBANKS_ - 1:
                            nc.tensor.matmul(
                                ps[:M, c0 : W_ - 1], m1s, xt[:k, c0 + 1 : W_],
                                start=False, stop=False,
                            )
                            nc.tensor.matmul(
                                ps[:M, W_ - 1 : W_], m1s, xt[:k, 0:1],
                                start=False, stop=True,
                            )
                        else:
                            nc.tensor.matmul(
                                ps[:M, c0:c1], m1s, xt[:k, c0 + 1 : c1 + 1],
                                start=False, stop=True,
                            )

                    # w = |around - 2.5|, split ScalarE / VectorE
                    wt = apool.tile([STRIDE_, W_], f16, tag="wt")
                    nc.scalar.activation(
                        wt[:M, 0:ABS_ACT_W], ps[:M, 0:ABS_ACT_W], AbsF,
                        bias=-2.5, scale=1.0,
                    )
                    nc.vector.tensor_scalar(
                        out=wt[:M, ABS_ACT_W:], in0=ps[:M, ABS_ACT_W:],
                        scalar1=-2.5, scalar2=0.0, op0=Add, op1=AbsMax,
                    )

                    # E2 = sigmoid(10*around - 25); E1 = sigmoid(10 - 10*w)
                    e2 = spool.tile([STRIDE_, W_], f16, tag="e2")
                    e1 = spool.tile([STRIDE_, W_], f16, tag="e1")
                    nc.scalar.activation(e2[:M], ps[:M], Sig, bias=bm25[:M], scale=10.0)
                    nc.scalar.activation(e1[:M], wt[:M], Sig, bias=bp10[:M], scale=-10.0)

                    # t = x + E2 - x*E2  (fused custom DVE op)
                    tt = tpool.tile([STRIDE_, W_], f16, tag="tt")
                    nc.vector._custom_dve(
                        blend1, out=tt[:M], in0=xt[:M, :], in1=e2[:M]
                    )

                    # out = E1 * t, split VectorE / GpSimd
                    o = opool.tile([STRIDE_, W_], f16, tag="o")
                    nc.vector.tensor_mul(
                        out=o[:M, 0:MUL_DVE_W], in0=e1[:M, 0:MUL_DVE_W],
                        in1=tt[:M, 0:MUL_DVE_W],
                    )
                    nc.gpsimd.tensor_mul(
                        out=o[:M, MUL_DVE_W:], in0=e1[:M, MUL_DVE_W:],
                        in1=tt[:M, MUL_DVE_W:],
                    )
                    nc.sync.dma_start(out=y_d[b, r0 : r0 + M, :], in_=o[:M])

    nc.compile()
    if (b_per, h, w, stride) == (B_PER, H, W, STRIDE):
        _cached_nc = nc
    return nc


def run(x, trace=False):
    """Run the SPMD kernel on 8 cores. Returns (out_fp32, BassKernelResults)."""
    from concourse.bass_utils import run_bass_kernel_spmd

    nc = _build()
    x = np.asarray(x, dtype=np.float32)
    assert x.shape == (B, H, W), x.shape
    in_maps = [{"x": x[B_PER * c : B_PER * (c + 1)]} for c in range(N_CORES)]
    res = run_bass_kernel_spmd(nc, in_maps, core_ids=list(range(N_CORES)), trace=trace)
    out = np.concatenate(
        [res.results[c]["y"].astype(np.float32) for c in range(N_CORES)], axis=0
    )
    return out, res


def kernel(x):
    out, _ = run(x, trace=False)
    return out


# revision 9
# speedup vs baseline: 1.0373x; 1.0373x over previous
"""Continuous Game-of-Life Trainium2 kernel (product-form, 2-sigmoid).

Reference computation (per batch image, cyclic 3x3 stencil):
    around = 8-neighbor sum of x (torus wrap), u = 10*around
    survive = sigmoid(u-15) * sigmoid(35-u)
    birth   = sigmoid(u-25) * sigmoid(35-u)
    out     = x*survive + (1-x)*birth

Math used here (max abs err ~5e-5 vs reference, fp64):
    E1 := sigmoid(10 - |u-25|)        # == survive (err <= sigmoid(-10))
    E2 := sigmoid(u-25)
    birth == E1*E2 (err ~5e-5), so
    out = E1 * (x + E2 - x*E2)

This needs only TWO sigmoid passes on the Scalar engine (the baseline
three-sigmoid form is ScalarE-bound at ~196us busy).  The remaining
work is spread to keep every engine under the ~4.4us/strip DMA floor:
  - TensorE: 8-neighbor sum via banded matmuls (as before).
  - abs pass w = |around-2.5|: split ScalarE (Abs activation, ~30%) /
    VectorE (tensor_scalar add+abs_max, ~70%; PSUM source runs 1x).
  - blend t = x + E2 - x*E2: one fused custom-DVE op (BLEND1_ANT).
  - out = E1*t: split VectorE (2x fp16) / GpSimd.
  - DMA in fp32->fp16 (SWDGE cast), out fp16.

Sharding: data-parallel over batch: 16 images -> 8 cores x 2 images.
Torus wrap is per-image so there is no cross-core halo.
"""

import numpy as np

B, H, W = 16, 2048, 2048
N_CORES = 8
B_PER = B // N_CORES  # 2 images per core
STRIDE = 126  # output rows per strip (128 input rows incl. halos)
N_STRIPS = (H + STRIDE - 1) // STRIDE  # 17
NBANKS = W // 512  # PSUM banks per strip

# work-split knobs (elements of the 2048-wide free dim)
ABS_ACT_W = 640  # abs columns done on ScalarE (rest on VectorE)
MUL_DVE_W = 1280  # final-mul columns done on VectorE (rest on GpSimd)
USE_CUSTOM_BLEND = True

_cached_nc = None
_custom_ops = None


def _register_custom_ops():
    """Register fused custom DVE ops at runtime.

    Same mechanism as editing dve_ops.py (the per-NEFF uop table is
    generated at compile time from OPS); the sha is computed here so the
    pin always matches this interpreter's lowering.

      BLEND1_ANT:    out = in0 + in1 - in0*in1
      ABS_SHIFT_ANT: out = |in0 + s0|   (walrus rejects abs_max on
                     TensorScalar, so plain TS cannot do an abs)
    """
    global _custom_ops
    if _custom_ops is not None:
        return _custom_ops
    import numpy as np

    from concourse import dve_ops
    from concourse.dve_spec import C0, Spec, Src0, Src1, Zero, lower, maxx
    from concourse.dve_uop import DveOpSpec

    def _mk(name, spec):
        if name in dve_ops._SUB_OPCODE_FOR_NAME:
            return next(op for op in dve_ops.OPS if op.name == name)
        shas = {
            ver: DveOpSpec(
                name=name, opcode=0, uops=lower(spec, ver=ver), rd1_en=True
            ).sha(ver)
            for ver in ("v3", "v4")
        }
        op = dve_ops.DveOp(name, spec, subdim=False, uops_sha=shas)
        row = dve_ops._CUSTOM_DVE_ROW_BASE + len(dve_ops.OPS)
        assert row < 0x20
        dve_ops.OPS.append(op)
        dve_ops._SUB_OPCODE_FOR_NAME[name] = row
        dve_ops.CUSTOM_DVE_SPECS[name] = spec
        return op

    blend = _mk(
        "BLEND1_ANT",
        Spec(
            body=Src0 + Src1 - Src0 * Src1,
            reference=lambda in0, in1, s0, s1, imm2: in0 + in1 - in0 * in1,
        ),
    )
    _y = Src0 + C0
    absshift = _mk(
        "ABS_SHIFT_ANT",
        Spec(
            body=maxx(_y, Zero - _y),
            reference=lambda in0, in1, s0, s1, imm2: np.abs(in0 + s0),
        ),
    )
    _custom_ops = (blend, absshift)
    return _custom_ops


def _band_matrices(m, dtype=np.float16):
    """[m+2, m] stationary operands for the vertical taps.

    Tile layout: partitions 0..m-1 hold image rows r0..r0+m-1 (the cells),
    partition m holds the bottom halo row r0+m, partition m+1 holds the top
    halo row r0-1.  For output row p the vertical neighbors are partitions
    p-1 (or m+1 when p==0) and p+1.

    m0[k, p] = 1 for the two vertical neighbors (no center),
    m1[k, p] = 1 for the full 3-tap (used on the column-shifted views).
    """
    m0 = np.zeros((m + 2, m), dtype)
    m1 = np.zeros((m + 2, m), dtype)
    for p in range(m):
        up = m + 1 if p == 0 else p - 1
        m0[up, p] = 1.0
        m0[p + 1, p] = 1.0
        m1[up, p] = 1.0
        m1[p, p] = 1.0
        m1[p + 1, p] = 1.0
    return m0, m1


def _build(b_per=B_PER, h=H, w=W, stride=STRIDE):
    global _cached_nc
    if _cached_nc is not None and (b_per, h, w, stride) == (B_PER, H, W, STRIDE):
        return _cached_nc

    import concourse.mybir as mybir
    from concourse.bacc import Bacc
    from concourse.tile import TileContext

    blend1, absshift = _register_custom_ops()

    B_PER_, H_, W_, STRIDE_ = b_per, h, w, stride
    N_STRIPS_ = (H_ + STRIDE_ - 1) // STRIDE_
    NBANKS_ = W_ // 512
    KROWS = STRIDE_ + 2

    f32 = mybir.dt.float32
    f16 = mybir.dt.float16
    Sig = mybir.ActivationFunctionType.Sigmoid
    AbsF = mybir.ActivationFunctionType.Abs
    Add = mybir.AluOpType.add
    AbsMax = mybir.AluOpType.abs_max

    nc = Bacc(trn_type="TRN2")
    x_d = nc.dram_tensor("x", [B_PER_, H_, W_], f32, kind="ExternalInput")
    y_d = nc.dram_tensor("y", [B_PER_, H_, W_], f16, kind="ExternalOutput")

    consts = {}
    for m in sorted({STRIDE_, H_ - STRIDE_ * (N_STRIPS_ - 1)}):
        m0_np, m1_np = _band_matrices(m)
        consts[m] = (
            nc.inline_tensor(m0_np, f"m0_const_{m}"),
            nc.inline_tensor(m1_np, f"m1_const_{m}"),
        )

    with TileContext(nc) as tc:
        with (
            tc.tile_pool(name="wpool", bufs=1) as wpool,
            tc.tile_pool(name="xpool", bufs=6) as xpool,
            tc.tile_pool(name="apool", bufs=4) as apool,
            tc.tile_pool(name="spool", bufs=4) as spool,
            tc.tile_pool(name="tpool", bufs=4) as tpool,
            tc.tile_pool(name="opool", bufs=6) as opool,
            tc.tile_pool(name="ppool", bufs=2, space="PSUM") as ppool,
        ):
            bands = {}
            for m, (m0_d, m1_d) in consts.items():
                m0 = wpool.tile([m + 2, m], f16, name=f"m0_{m}")
                m1 = wpool.tile([m + 2, m], f16, name=f"m1_{m}")
                nc.sync.dma_start(out=m0[:], in_=m0_d[:])
                nc.sync.dma_start(out=m1[:], in_=m1_d[:])
                bands[m] = (m0, m1)

            # activation biases must be [128,1] APs, not immediates
            bm25 = wpool.tile([128, 1], f32)
            bp10 = wpool.tile([128, 1], f32)
            bm2p5 = wpool.tile([128, 1], f32)
            nc.vector.memset(bm25[:], -25.0)
            nc.vector.memset(bp10[:], 10.0)
            nc.vector.memset(bm2p5[:], -2.5)

            for b in range(B_PER_):
                for t in range(N_STRIPS_):
                    r0 = t * STRIDE_
                    M = min(STRIDE_, H_ - r0)  # output rows this strip
                    k = M + 2
                    m0, m1 = bands[M]

                    # fp16 tile, partitions 0..M-1 = cells (rows r0..),
                    # partition M = bottom halo, M+1 = top halo.  gpsimd
                    # (SWDGE) DMA casts fp32->fp16 in flight.
                    xt = xpool.tile([KROWS, W_], f16, tag="xt")
                    if r0 + M < H_:
                        nc.gpsimd.dma_start(
                            out=xt[0 : M + 1, :], in_=x_d[b, r0 : r0 + M + 1, :]
                        )
                    else:
                        # last strip: bottom halo wraps to row 0
                        nc.gpsimd.dma_start(out=xt[0:M, :], in_=x_d[b, r0:H_, :])
                        nc.gpsimd.dma_start(out=xt[M : M + 1, :], in_=x_d[b, 0:1, :])
                    rtop = (r0 - 1) % H_
                    nc.gpsimd.dma_start(
                        out=xt[M + 1 : M + 2, :], in_=x_d[b, rtop : rtop + 1, :]
                    )

                    ps = ppool.tile([STRIDE_, W_], f32, tag="ps")
                    m0s = m0[:k, :M]
                    m1s = m1[:k, :M]

                    # Pre-touch: a 1x1 matmul absorbs the PSUM-release wait
                    # (Matmult carries at most ONE sync wait; without this,
                    # wait-merging couples strip t to strip t-1's consumers
                    # and serializes PE behind ACT/DVE).
                    nc.tensor.matmul(
                        ps[:1, 0:1], bm25[:1, :1], bm25[:1, :1],
                        start=True, stop=True,
                    )

                    # around = sum of 8 neighbors, accumulated in PSUM.
                    for nb in range(NBANKS_):
                        c0 = nb * 512
                        c1 = c0 + 512
                        # center column, vertical neighbors only
                        nc.tensor.matmul(
                            ps[:M, c0:c1], m0s, xt[:k, c0:c1],
                            start=True, stop=False,
                        )
                        # left-neighbor column: out col j += band @ x col j-1
                        if nb == 0:
                            nc.tensor.matmul(
                                ps[:M, 1:512], m1s, xt[:k, 0:511],
                                start=False, stop=False,
                            )
                            nc.tensor.matmul(
                                ps[:M, 0:1], m1s, xt[:k, W_ - 1 : W_],
                                start=False, stop=False,
                            )
                        else:
                            nc.tensor.matmul(
                                ps[:M, c0:c1], m1s, xt[:k, c0 - 1 : c1 - 1],
                                start=False, stop=False,
                            )
                        # right-neighbor column: out col j += band @ x col j+1
                        if nb == NBANKS_ - 1:
                            nc.tensor.matmul(
                                ps[:M, c0 : W_ - 1], m1s, xt[:k, c0 + 1 : W_],
                                start=False, stop=False,
                            )
                            nc.tensor.matmul(
                                ps[:M, W_ - 1 : W_], m1s, xt[:k, 0:1],
                                start=False, stop=True,
                            )
                        else:
                            nc.tensor.matmul(
                                ps[:M, c0:c1], m1s, xt[:k, c0 + 1 : c1 + 1],
                                start=False, stop=True,
                            )

                    # w = |around - 2.5|, split ScalarE / VectorE
                    wt = apool.tile([STRIDE_, W_], f16, tag="wt")
                    nc.scalar.activation(
                        wt[:M, 0:ABS_ACT_W], ps[:M, 0:ABS_ACT_W], AbsF,
                        bias=bm2p5[:M], scale=1.0,
                    )
                    nc.vector._custom_dve(
                        absshift, out=wt[:M, ABS_ACT_W:], in0=ps[:M, ABS_ACT_W:],
                        s0=-2.5,
                    )

                    # E2 = sigmoid(10*around - 25); E1 = sigmoid(10 - 10*w)
                    e2 = spool.tile([STRIDE_, W_], f16, tag="e2")
                    e1 = spool.tile([STRIDE_, W_], f16, tag="e1")
                    nc.scalar.activation(e2[:M], ps[:M], Sig, bias=bm25[:M], scale=10.0)
                    nc.scalar.activation(e1[:M], wt[:M], Sig, bias=bp10[:M], scale=-10.0)

                    # t = x + E2 - x*E2  (fused custom DVE op)
                    tt = tpool.tile([STRIDE_, W_], f16, tag="tt")
                    if USE_CUSTOM_BLEND:
                        nc.vector._custom_dve(
                            blend1, out=tt[:M], in0=xt[:M, :], in1=e2[:M]
                        )
                    else:
                        mm = tpool.tile([STRIDE_, W_], f16, tag="mm")
                        nc.vector.tensor_mul(out=mm[:M], in0=xt[:M, :], in1=e2[:M])
                        nc.vector.tensor_sub(out=mm[:M], in0=e2[:M], in1=mm[:M])
                        nc.vector.tensor_add(out=tt[:M], in0=xt[:M, :], in1=mm[:M])

                    # out = E1 * t, split VectorE / GpSimd
                    o = opool.tile([STRIDE_, W_], f16, tag="o")
                    nc.vector.tensor_mul(
                        out=o[:M, 0:MUL_DVE_W], in0=e1[:M, 0:MUL_DVE_W],
                        in1=tt[:M, 0:MUL_DVE_W],
                    )
                    nc.gpsimd.tensor_mul(
                        out=o[:M, MUL_DVE_W:], in0=e1[:M, MUL_DVE_W:],
                        in1=tt[:M, MUL_DVE_W:],
                    )
                    nc.sync.dma_start(out=y_d[b, r0 : r0 + M, :], in_=o[:M])

    nc.compile()
    if (b_per, h, w, stride) == (B_PER, H, W, STRIDE):
        _cached_nc = nc
    return nc


def run(x, trace=False):
    """Run the SPMD kernel on 8 cores. Returns (out_fp32, BassKernelResults)."""
    from concourse.bass_utils import run_bass_kernel_spmd

    nc = _build()
    x = np.asarray(x, dtype=np.float32)
    assert x.shape == (B, H, W), x.shape
    in_maps = [{"x": x[B_PER * c : B_PER * (c + 1)]} for c in range(N_CORES)]
    res = run_bass_kernel_spmd(nc, in_maps, core_ids=list(range(N_CORES)), trace=trace)
    out = np.concatenate(
        [res.results[c]["y"].astype(np.float32) for c in range(N_CORES)], axis=0
    )
    return out, res


def kernel(x):
    out, _ = run(x, trace=False)
    return out
